# revision 12
# baseline (speedup 1.0000x reference)
"""AncProbsLayer on 8 TRN2 NeuronCores.

Structure of the problem: tauQ[m,b,k] = mut_rates[m,b,k] * Q[m,k], so the
expm inputs are scalar multiples of only m*k tiny rate matrices, and
P[m,b,k] = expm(tauQ) is (m,b,k,20,20) ~= 13MB -- cheap to compute exactly
on the host. The heavy part (by IO and FLOPs) is the batched einsum
    out[m,b] = A[m,b] @ concat_k P[m,b,k]      (1024,20)@(20,80) per pair,
which runs on the 8 cores, data-parallel over b. Four (m,b) pairs are
stacked block-diagonally per matmul (K=4*20=80 partitions, N=4*80=320
free) so the PE array is well utilized; compute dtype is bf16 (tolerance
is loose), halving DMA traffic vs f32.
"""

import numpy as np
import ml_dtypes

import concourse.bass as bass
import concourse.mybir as mybir
from concourse.tile import TileContext
from concourse.bass_utils import run_bass_kernel_spmd

S = 20          # amino acids
M_ = 2          # models
B = 256         # sequence batch
L = 1024        # sequence length
K = 4           # matrices per model
N_CORES = 8
BS = B // N_CORES          # 32 sequences per core
PAIRS = M_ * BS            # 64 (m,b) pairs per core
GP = 4                     # pairs stacked block-diagonally per matmul
G = PAIRS // GP            # 16 groups per core
KDIM = GP * S              # 80 contraction partitions
NDIM = GP * K * S          # 320 psum free columns
CH = L // 128              # 8 row chunks of 128
NQ = 8                     # output DMA queues (one big DMA per queue)
GPQ = G // NQ              # groups staged per queue

BF16 = mybir.dt.bfloat16
NPBF16 = ml_dtypes.bfloat16

TRACE = False
TRACE_DIR = None
LAST = {"exec_time_ns": None}
_NC_CACHE = {}


def _install_trace_shims():
    """Test-only: register the NTFF profile hook (missing from this image's
    antenv) and defang the artifact upload so trace=True works locally."""
    import sys as _sys
    import types as _types

    try:
        from antenv.axon_hooks import get_axon_ntff_profile_hook  # noqa: F401
    except ImportError:
        from trn_agent_boot.trn_boot import _ntff_profile_via_ctypes

        hook = _ntff_profile_via_ctypes("/opt/axon/libaxon_pjrt.so")
        mod = _types.ModuleType("antenv.axon_hooks")
        mod.get_axon_ntff_profile_hook = lambda: hook
        mod.set_axon_ntff_profile_hook = lambda h: None
        _sys.modules["antenv.axon_hooks"] = mod

    import concourse.bass_utils as bu

    bu.upload_artifacts = lambda tmpdir: str(tmpdir)


def _softplus(x):
    return np.logaddexp(0.0, x)


def _host_pcat(tau_kernel, exchangeability_kernel, equilibrium_kernel,
               per_matrix_rates_kernel, rate_indices):
    """(m,b,S,K*S) float32: per-(m,b) transition matrices, concatenated over k."""
    tk = np.asarray(tau_kernel, dtype=np.float64)
    ek = np.asarray(exchangeability_kernel, dtype=np.float64)
    qk = np.asarray(equilibrium_kernel, dtype=np.float64)
    pk = np.asarray(per_matrix_rates_kernel, dtype=np.float64)
    idx = np.asarray(rate_indices, dtype=np.int64)

    tau = _softplus(np.take_along_axis(tk, idx, axis=1))           # (m,b)
    pmr = _softplus(pk)                                            # (m,k)
    mut = tau[:, :, None] * pmr[:, None, :]                        # (m,b,k)

    R = _softplus(0.5 * (ek + np.swapaxes(ek, -1, -2)))
    R = R * (1.0 - np.eye(S))                                      # (m,k,S,S)
    e = qk - qk.max(axis=-1, keepdims=True)
    p = np.exp(e)
    p /= p.sum(axis=-1, keepdims=True)                             # (m,k,S)

    Q = R * p[:, :, None, :]
    diag = Q.sum(axis=-1, keepdims=True)                           # (m,k,S,1)
    Q = Q - diag * np.eye(S)
    mue = np.sum(p[..., None] * diag, axis=-2, keepdims=True)      # (m,k,1,1)
    Q = Q / np.maximum(mue, 1e-16)

    A = mut[..., None, None] * Q[:, None]                          # (m,b,k,S,S)
    A = A / 64.0                                                   # 2^-6 scaling
    eye = np.broadcast_to(np.eye(S), A.shape)
    out = eye.copy()
    term = eye.copy()
    for i in range(1, 15):
        term = term @ A / i
        out = out + term
    for _ in range(6):
        out = out @ out
    # (m,b,k,z,s) -> (m,b,z,k*s)
    return out.transpose(0, 1, 3, 2, 4).reshape(M_, B, S, K * S).astype(np.float32)


def _split_multi_waits(nc):
    """walrus codegen on this toolchain supports one sync-wait slot per
    instruction; Tile's kernel-tail drain accumulates one wait per touched
    semaphore. Split extra waits onto single-wait NoOps on the same engine."""
    f = nc.m.functions[0]
    for blk in f.blocks:
        insts = blk.instructions
        i = 0
        while i < len(insts):
            inst = insts[i]
            si = getattr(inst, "sync_info", None)
            if si is not None and si.on_wait and len(si.on_wait) > 1:
                assert not isinstance(inst, mybir.InstDMACopy), (
                    "multi-wait DMA cannot be split onto its queue"
                )
                waits = list(si.on_wait)
                for w in waits[:-1]:
                    nop = mybir.InstNoOp(
                        name=nc.get_next_instruction_name(),
                        sync_info=mybir.SyncInfo(on_wait=[w], on_update=[]),
                        bass_nofuse=True,
                        engine=inst.engine,
                    )
                    nc.register_instruction(nop)
                    insts.insert(i, nop)
                    i += 1
                si.on_wait = [waits[-1]]
            i += 1


def _build_nc():
    if "nc" in _NC_CACHE:
        return _NC_CACHE["nc"]
    nc = bass.Bass()
    a_t = nc.declare_dram_parameter("a_t", [G, KDIM, L], BF16, isOutput=False)
    rhs = nc.declare_dram_parameter("rhs", [KDIM, G * NDIM], BF16, isOutput=False)
    # one contiguous (128, GPQ*CH*NDIM) region per output DMA queue
    out = nc.declare_dram_parameter(
        "out", [NQ, 128, GPQ * CH * NDIM], BF16, isOutput=True
    )

    with TileContext(nc) as tc:
        with (
            tc.tile_pool(name="ins", bufs=1) as ins,
            tc.tile_pool(name="st", bufs=1) as stp,
            tc.tile_pool(name="ps", bufs=8, space="PSUM") as ps,
        ):
            rhs_t = ins.tile([KDIM, G * NDIM], BF16, tag="rhs")
            nc.sync.dma_start(out=rhs_t[:], in_=rhs[:])
            at_tiles = []
            for g in range(G):
                t = ins.tile([KDIM, L], BF16, tag=f"at{g}", name=f"at{g}")
                nc.sync.dma_start(out=t[:], in_=a_t[g])
                at_tiles.append(t)
            st_tiles = [
                stp.tile([128, GPQ * CH * NDIM], BF16, tag=f"st{q}", name=f"st{q}")
                for q in range(NQ)
            ]
            for g in range(G):
                for c in range(CH):
                    pt = ps.tile([128, NDIM], mybir.dt.float32, tag="ps")
                    nc.tensor.matmul(
                        pt[:],
                        at_tiles[g][:, c * 128:(c + 1) * 128],
                        rhs_t[:, g * NDIM:(g + 1) * NDIM],
                        start=True,
                        stop=True,
                    )
                    col = ((g % GPQ) * CH + c) * NDIM
                    nc.vector.tensor_copy(
                        out=st_tiles[g // GPQ][:, col:col + NDIM], in_=pt[:]
                    )
            for q in range(NQ):
                nc.gpsimd.dma_start(out=out[q], in_=st_tiles[q][:])
    _split_multi_waits(nc)
    _NC_CACHE["nc"] = nc
    return nc


def kernel(inputs, tau_kernel, exchangeability_kernel, equilibrium_kernel,
           per_matrix_rates_kernel, rate_indices):
    inputs = np.asarray(inputs)
    pcat = _host_pcat(tau_kernel, exchangeability_kernel, equilibrium_kernel,
                      per_matrix_rates_kernel, rate_indices)

    in_maps = []
    for core in range(N_CORES):
        bsl = slice(core * BS, (core + 1) * BS)
        a = inputs[:, bsl].reshape(PAIRS, L, S).transpose(0, 2, 1)   # (64,S,L)
        a = np.ascontiguousarray(a).reshape(G, KDIM, L).astype(NPBF16)
        pc = pcat[:, bsl].reshape(G, GP, S, K * S)
        rhs = np.zeros((G, KDIM, NDIM), np.float32)
        for i in range(GP):
            rhs[:, i * S:(i + 1) * S, i * K * S:(i + 1) * K * S] = pc[:, i]
        rhs = np.ascontiguousarray(rhs.transpose(1, 0, 2)).reshape(KDIM, G * NDIM)
        in_maps.append({"a_t": a, "rhs": rhs.astype(NPBF16)})

    nc = _build_nc()
    if TRACE:
        _install_trace_shims()
        res = run_bass_kernel_spmd(nc, in_maps, list(range(N_CORES)),
                                   trace=True, tmpdir=TRACE_DIR)
    else:
        res = run_bass_kernel_spmd(nc, in_maps, list(range(N_CORES)))
    LAST["exec_time_ns"] = res.exec_time_ns

    full = np.empty((M_, B, L, K * S), np.float32)
    for core in range(N_CORES):
        bsl = slice(core * BS, (core + 1) * BS)
        r = np.asarray(res.results[core]["out"])      # (NQ,128,GPQ*CH*NDIM)
        # columns per queue are (gg, c, i, j); partitions are l within chunk c
        r = r.reshape(NQ, 128, GPQ, CH, GP, K * S).transpose(0, 2, 4, 3, 1, 5)
        # -> (NQ, GPQ, GP, CH, 128, K*S); pair index = (q*GPQ+gg)*GP+i
        full[:, bsl] = r.reshape(M_, BS, L, K * S).astype(np.float32)
    return full


# revision 13
# speedup vs baseline: 1.1385x; 1.1385x over previous
"""AncProbsLayer on 8 TRN2 NeuronCores.

Structure of the problem: tauQ[m,b,k] = mut_rates[m,b,k] * Q[m,k], so the
expm inputs are scalar multiples of only m*k tiny rate matrices, and
P[m,b,k] = expm(tauQ) is (m,b,k,20,20) ~= 13MB -- cheap to compute exactly
on the host. The heavy part (by IO and FLOPs) is the batched einsum
    out[m,b] = A[m,b] @ concat_k P[m,b,k]      (1024,20)@(20,80) per pair,
which runs on the 8 cores, data-parallel over b. Four (m,b) pairs are
stacked block-diagonally per matmul (K=4*20=80 partitions, N=4*80=320
free) so the PE array is well utilized; compute dtype is bf16 (tolerance
is loose), halving DMA traffic vs f32.
"""

import numpy as np
import ml_dtypes

import concourse.bass as bass
import concourse.mybir as mybir
from concourse.tile import TileContext
from concourse.bass_utils import run_bass_kernel_spmd

S = 20          # amino acids
M_ = 2          # models
B = 256         # sequence batch
L = 1024        # sequence length
K = 4           # matrices per model
N_CORES = 8
BS = B // N_CORES          # 32 sequences per core
PAIRS = M_ * BS            # 64 (m,b) pairs per core
GP = 4                     # pairs stacked block-diagonally per matmul
G = PAIRS // GP            # 16 groups per core
KDIM = GP * S              # 80 contraction partitions
NDIM = GP * K * S          # 320 psum free columns
CH = L // 128              # 8 row chunks of 128
NQ = 8                     # output DMA queues (one big DMA per queue)
GPQ = G // NQ              # groups staged per queue

BF16 = mybir.dt.bfloat16
NPBF16 = ml_dtypes.bfloat16

TRACE = False
TRACE_DIR = None
LAST = {"exec_time_ns": None}
_NC_CACHE = {}


def _install_trace_shims():
    """Test-only: register the NTFF profile hook (missing from this image's
    antenv) and defang the artifact upload so trace=True works locally."""
    import sys as _sys
    import types as _types

    try:
        from antenv.axon_hooks import get_axon_ntff_profile_hook  # noqa: F401
    except ImportError:
        from trn_agent_boot.trn_boot import _ntff_profile_via_ctypes

        hook = _ntff_profile_via_ctypes("/opt/axon/libaxon_pjrt.so")
        mod = _types.ModuleType("antenv.axon_hooks")
        mod.get_axon_ntff_profile_hook = lambda: hook
        mod.set_axon_ntff_profile_hook = lambda h: None
        _sys.modules["antenv.axon_hooks"] = mod

    import concourse.bass_utils as bu

    bu.upload_artifacts = lambda tmpdir: str(tmpdir)


def _softplus(x):
    return np.logaddexp(0.0, x)


def _host_pcat(tau_kernel, exchangeability_kernel, equilibrium_kernel,
               per_matrix_rates_kernel, rate_indices):
    """(m,b,S,K*S) float32: per-(m,b) transition matrices, concatenated over k."""
    tk = np.asarray(tau_kernel, dtype=np.float64)
    ek = np.asarray(exchangeability_kernel, dtype=np.float64)
    qk = np.asarray(equilibrium_kernel, dtype=np.float64)
    pk = np.asarray(per_matrix_rates_kernel, dtype=np.float64)
    idx = np.asarray(rate_indices, dtype=np.int64)

    tau = _softplus(np.take_along_axis(tk, idx, axis=1))           # (m,b)
    pmr = _softplus(pk)                                            # (m,k)
    mut = tau[:, :, None] * pmr[:, None, :]                        # (m,b,k)

    R = _softplus(0.5 * (ek + np.swapaxes(ek, -1, -2)))
    R = R * (1.0 - np.eye(S))                                      # (m,k,S,S)
    e = qk - qk.max(axis=-1, keepdims=True)
    p = np.exp(e)
    p /= p.sum(axis=-1, keepdims=True)                             # (m,k,S)

    Q = R * p[:, :, None, :]
    diag = Q.sum(axis=-1, keepdims=True)                           # (m,k,S,1)
    Q = Q - diag * np.eye(S)
    mue = np.sum(p[..., None] * diag, axis=-2, keepdims=True)      # (m,k,1,1)
    Q = Q / np.maximum(mue, 1e-16)

    A = mut[..., None, None] * Q[:, None]                          # (m,b,k,S,S)
    A = A / 64.0                                                   # 2^-6 scaling
    eye = np.broadcast_to(np.eye(S), A.shape)
    out = eye.copy()
    term = eye.copy()
    for i in range(1, 15):
        term = term @ A / i
        out = out + term
    for _ in range(6):
        out = out @ out
    # (m,b,k,z,s) -> (m,b,z,k*s)
    return out.transpose(0, 1, 3, 2, 4).reshape(M_, B, S, K * S).astype(np.float32)


def _split_multi_waits(nc):
    """walrus codegen on this toolchain supports one sync-wait slot per
    instruction; Tile's kernel-tail drain accumulates one wait per touched
    semaphore. Split extra waits onto single-wait NoOps on the same engine."""
    f = nc.m.functions[0]
    for blk in f.blocks:
        insts = blk.instructions
        i = 0
        while i < len(insts):
            inst = insts[i]
            si = getattr(inst, "sync_info", None)
            if si is not None and si.on_wait and len(si.on_wait) > 1:
                assert not isinstance(inst, mybir.InstDMACopy), (
                    "multi-wait DMA cannot be split onto its queue"
                )
                waits = list(si.on_wait)
                for w in waits[:-1]:
                    nop = mybir.InstNoOp(
                        name=nc.get_next_instruction_name(),
                        sync_info=mybir.SyncInfo(on_wait=[w], on_update=[]),
                        bass_nofuse=True,
                        engine=inst.engine,
                    )
                    nc.register_instruction(nop)
                    insts.insert(i, nop)
                    i += 1
                si.on_wait = [waits[-1]]
            i += 1


def _build_nc():
    if "nc" in _NC_CACHE:
        return _NC_CACHE["nc"]
    nc = bass.Bass()
    a_t = nc.declare_dram_parameter("a_t", [G, KDIM, L], BF16, isOutput=False)
    rhs = nc.declare_dram_parameter("rhs", [KDIM, G * NDIM], BF16, isOutput=False)
    # one contiguous (128, GPQ*CH*NDIM) region per output DMA queue
    out = nc.declare_dram_parameter(
        "out", [NQ, 128, GPQ * CH * NDIM], BF16, isOutput=True
    )

    with TileContext(nc) as tc:
        with (
            tc.tile_pool(name="ins", bufs=1) as ins,
            tc.tile_pool(name="st", bufs=1) as stp,
            tc.tile_pool(name="ps", bufs=8, space="PSUM") as ps,
        ):
            rhs_t = ins.tile([KDIM, G * NDIM], BF16, tag="rhs")
            nc.sync.dma_start(out=rhs_t[:], in_=rhs[:])
            at_tiles = []
            for g in range(G):
                t = ins.tile([KDIM, L], BF16, tag=f"at{g}", name=f"at{g}")
                nc.sync.dma_start(out=t[:], in_=a_t[g])
                at_tiles.append(t)
            st_tiles = [
                stp.tile([128, GPQ * CH * NDIM], BF16, tag=f"st{q}", name=f"st{q}")
                for q in range(NQ)
            ]
            # Queues 0-3 (groups 0-7) are cast on DVE, queues 4-7 (groups
            # 8-15) on ScalarE, so each staging tile has one producer engine
            # and each output DMA needs exactly one wait. Emission alternates
            # the two engines' (g,c) tiles so both cast streams stay busy.
            dve_work = [(g, c) for g in range(G // 2) for c in range(CH)]
            act_work = [(g, c) for g in range(G // 2, G) for c in range(CH)]
            for i in range(G * CH):
                g, c = dve_work[i // 2] if i % 2 == 0 else act_work[i // 2]
                pt = ps.tile([128, NDIM], mybir.dt.float32, tag="ps")
                nc.tensor.matmul(
                    pt[:],
                    at_tiles[g][:, c * 128:(c + 1) * 128],
                    rhs_t[:, g * NDIM:(g + 1) * NDIM],
                    start=True,
                    stop=True,
                )
                col = ((g % GPQ) * CH + c) * NDIM
                dst = st_tiles[g // GPQ][:, col:col + NDIM]
                if i % 2 == 0:
                    nc.vector.tensor_copy(out=dst, in_=pt[:])
                else:
                    nc.scalar.copy(out=dst, in_=pt[:])
            for q in range(NQ):
                nc.gpsimd.dma_start(out=out[q], in_=st_tiles[q][:])
    _split_multi_waits(nc)
    _NC_CACHE["nc"] = nc
    return nc


def kernel(inputs, tau_kernel, exchangeability_kernel, equilibrium_kernel,
           per_matrix_rates_kernel, rate_indices):
    inputs = np.asarray(inputs)
    pcat = _host_pcat(tau_kernel, exchangeability_kernel, equilibrium_kernel,
                      per_matrix_rates_kernel, rate_indices)

    in_maps = []
    for core in range(N_CORES):
        bsl = slice(core * BS, (core + 1) * BS)
        a = inputs[:, bsl].reshape(PAIRS, L, S).transpose(0, 2, 1)   # (64,S,L)
        a = np.ascontiguousarray(a).reshape(G, KDIM, L).astype(NPBF16)
        pc = pcat[:, bsl].reshape(G, GP, S, K * S)
        rhs = np.zeros((G, KDIM, NDIM), np.float32)
        for i in range(GP):
            rhs[:, i * S:(i + 1) * S, i * K * S:(i + 1) * K * S] = pc[:, i]
        rhs = np.ascontiguousarray(rhs.transpose(1, 0, 2)).reshape(KDIM, G * NDIM)
        in_maps.append({"a_t": a, "rhs": rhs.astype(NPBF16)})

    nc = _build_nc()
    if TRACE:
        _install_trace_shims()
        res = run_bass_kernel_spmd(nc, in_maps, list(range(N_CORES)),
                                   trace=True, tmpdir=TRACE_DIR)
    else:
        res = run_bass_kernel_spmd(nc, in_maps, list(range(N_CORES)))
    LAST["exec_time_ns"] = res.exec_time_ns

    full = np.empty((M_, B, L, K * S), np.float32)
    for core in range(N_CORES):
        bsl = slice(core * BS, (core + 1) * BS)
        r = np.asarray(res.results[core]["out"])      # (NQ,128,GPQ*CH*NDIM)
        # columns per queue are (gg, c, i, j); partitions are l within chunk c
        r = r.reshape(NQ, 128, GPQ, CH, GP, K * S).transpose(0, 2, 4, 3, 1, 5)
        # -> (NQ, GPQ, GP, CH, 128, K*S); pair index = (q*GPQ+gg)*GP+i
        full[:, bsl] = r.reshape(M_, BS, L, K * S).astype(np.float32)
    return full
